# revision 7
# baseline (speedup 1.0000x reference)
"""Trainium2 Bass kernel for SimCLR-style contrastive (NT-Xent) loss.

Reference computation:
    z = concat(emb_i, emb_j)            # [8192, 256]
    z = z / ||z||_row
    sim = (z @ z.T) / 0.5               # [8192, 8192]
    sim[i, i] = -inf
    loss = mean_i( logsumexp_j(sim[i, :]) - sim[i, label_i] )
    label_i = (i + 4096) % 8192

v2 design: symmetric half-Gram + fp8 DoubleRow matmuls.

Distribution: data-parallel over rows, 1024 rows (8 tiles of 128) per
core; host pre-rotates z per core (np.roll = pure resharding) so the
SPMD program is identical on all cores. exp(sim) is symmetric, so each
core computes Gram tiles (t, u) only for u in [t, t+32]: every global
unordered tile pair with separation d in [0,32) is computed exactly once
across cores; d=32 pairs are computed twice (once per owning core,
row-sums only). Row-sums of the computed tiles serve the core's own 8
row-tiles; COLUMN-sums of the d in [1,32) tiles provide the "behind"
halves of rows owned by other cores. Per-core partial sums are combined
on the host (O(N) epilogue: scatter-add + log + mean).

Each core therefore only touches local row tiles [0, 40): loads 5120
rows (5.25 MB) instead of the full 8 MB.

Numerics: rows are normalized and scaled by 16 (components ~N(0,1)),
quantized to fp8e4m3. Adjacent fp8 pairs are bitcast to uint16 for the
DMA-xbar transpose (2-byte min dtype); the resulting [d_pair, row]
layout with byte index = d parity is directly a valid fp8 DoubleRow
matmul operand pair (contraction k = 2*partition + byte on BOTH sides,
and any bijection (partition,subtile)->d gives the same dot product).
One DoubleRow matmul contracts all K=256 at 0.5 cycles/col. PSUM holds
256*sim; exp runs on ScalarE with scale=1/128, bias=-2 (sim<=1 so the
shift is a safe softmax max-shift), fused row-sum accumulation, bf16
tile output. DVE adds e-tiles into 39 absolute-column buckets, one
batched DMA-xbar transpose turns the buckets row-major, a reduce
finishes the column sums.
"""

import os
import sys
from contextlib import ExitStack

import numpy as np

for _p in ("/opt/trn_rl_repo",):
    if os.path.isdir(_p) and _p not in sys.path:
        sys.path.insert(0, _p)

import concourse.bacc as bacc
import concourse.tile as tile
from concourse import mybir
from concourse.bass_utils import run_bass_kernel_spmd

F32 = mybir.dt.float32
BF16 = mybir.dt.bfloat16
FP8 = mybir.dt.float8e4
U16 = mybir.dt.uint16
AF = mybir.ActivationFunctionType
ALU = mybir.AluOpType
DR = mybir.MatmulPerfMode.DoubleRow

N, D = 8192, 256          # 2B rows, feature dim
NCORES = 8
ROWS = N // NCORES        # 1024 rows owned per core
RT = ROWS // 128          # 8 own row-tiles
UT = 40                   # loaded row-tiles (strip: own + 32 ahead)
SR = UT * 128             # 5120 strip rows
CH = 1408                 # exp chunk width (3 chunks cover 33 tiles)
NCH = 3
NG = UT // 8              # 5 load groups of 8 tiles
LN16 = float(np.log(16.0))
QSCALE = 1.0 / 128.0      # psum = 256*sim ; exp arg = 2*sim - 2

# ---- engine assignment knobs (tuned from traces) ----
# fraction of bucket adds on gpsimd: set of t values
ADDS_GPS_T = set()
# sumsq tiles on ACT instead of DVE: t8 positions within each group
SUMSQ_ACT_T8 = set()

_ACT_SET = "natural_log_exp_and_others"   # contains exp, ln, square, copy


def _patch_act_tables():
    """Restrict the ACT table-set chooser to the one set containing every
    function this kernel uses, avoiding ACT_TABLE_LOAD churn."""
    if getattr(bacc, "_act_tables_patched", False):
        return
    orig = bacc.get_activation_tables

    def restricted(arch):
        full = dict(orig(arch))
        return {
            name: (fns if name == _ACT_SET else set())
            for name, fns in full.items()
        }

    bacc.get_activation_tables = restricted
    bacc._act_tables_patched = True


def _build_kernel(ctx, tc, z, outs, outc):
    nc = tc.nc
    zr = z.rearrange("(a p) d -> p a d", p=128)  # [128, 40, 256] DRAM view

    staging = ctx.enter_context(tc.tile_pool(name="staging", bufs=3))
    epool = ctx.enter_context(tc.tile_pool(name="epool", bufs=8))
    ppool = ctx.enter_context(tc.tile_pool(name="ppool", bufs=2, space="PSUM"))
    persist = ctx.enter_context(tc.tile_pool(name="persist", bufs=1))

    zq = persist.tile([128, UT, D], FP8)      # normalized rows * 16, byte-
    # permuted so byte o of tile t holds d = (o%2)*128 + o//2: the u16 atom
    # dd then holds (d=dd, d=dd+128), which after the u16 DMA transpose is
    # exactly the dual-fp8 ifmap pairing (k=p, k=p+128) the PE streams.
    zTp = persist.tile([128, SR], U16)        # packed transpose (fp8 pairs)
    wpm = persist.tile([128, 2, 1024], FP8)   # plane-major weights (own rows)
    ss = persist.tile([128, UT], F32)
    lss = persist.tile([128, UT], F32)
    rinv16 = persist.tile([128, UT], F32)     # 16 / ||z_row||
    sparts = persist.tile([128, RT * NCH], F32)
    qss = persist.tile([128, RT], F32)        # 256*sim_ii (quantized rows)
    acc = persist.tile([128, 39 * 128], BF16)  # colsum buckets u=1..39
    accT = persist.tile([128, 39, 128], BF16)
    junk_dve = persist.tile([128, D], F32)
    junk_act = persist.tile([128, D], F32)
    junk_dot = persist.tile([128, D], F32)
    Sown = persist.tile([128, RT], F32)
    ediag = persist.tile([128, RT], F32)
    outs_sb = persist.tile([128, 16], F32)    # [0:8]=S_own-ediag, [8:16]=pdot
    outc_sb = persist.tile([128, 39], F32)
    negtwo = persist.tile([128, 1], F32)
    ln16 = persist.tile([128, 1], F32)
    nc.vector.memset(negtwo[:], -2.0)
    nc.vector.memset(ln16[:], LN16)

    nc.vector.memset(acc[:], 0.0)

    zq_u16 = zq[:].bitcast(U16)               # [128, 40, 128]
    zT8 = zTp[:].bitcast(FP8).rearrange("p (r b) -> p b r", b=2)  # [128,2,5120]

    # ---- phase A: load + sumsq + rinv + quantize + transpose ----
    def head(g):
        st = staging.tile([128, 8, D], F32, tag="st", name="st")
        for q in range(4):
            nc.sync.dma_start(
                st[:, q * 2:(q + 1) * 2, :],
                zr[:, g * 8 + q * 2:g * 8 + (q + 1) * 2, :],
            )
        for t8 in range(8):
            t = g * 8 + t8
            if t8 in SUMSQ_ACT_T8:
                nc.scalar.activation(
                    junk_act[:], st[:, t8, :], AF.Square,
                    accum_out=ss[:, t:t + 1],
                )
            else:
                nc.vector.scalar_tensor_tensor(
                    out=junk_dve[:], in0=st[:, t8, :], scalar=1.0,
                    in1=st[:, t8, :], op0=ALU.mult, op1=ALU.mult,
                    accum_out=ss[:, t:t + 1],
                )
        gsl = slice(g * 8, (g + 1) * 8)
        nc.scalar.activation(lss[:, gsl], ss[:, gsl], AF.Ln)
        # 16 * ss^-0.5 = exp(-0.5*ln(ss) + ln 16)
        nc.scalar.activation(rinv16[:, gsl], lss[:, gsl], AF.Exp,
                             scale=-0.5, bias=ln16[:, 0:1])
        return st

    def tail(g, st):
        for t8 in range(8):
            t = g * 8 + t8
            # byte-permuted write: zq[p, t, 2*dd+b] = zn[p, b*128+dd] * rinv
            ov = zq[:, t, :].rearrange("p (dd b) -> p b dd", b=2)
            iv = st[:, t8, :].rearrange("p (b dd) -> p b dd", b=2)
            nc.vector.tensor_scalar_mul(ov, iv, rinv16[:, t:t + 1])
        o3 = zTp[:, g * 1024:(g + 1) * 1024].rearrange(
            "p (t c) -> p t c", c=128
        )
        nc.sync.dma_start_transpose(o3, zq_u16[:, g * 8:(g + 1) * 8, :])
        if g == 0:
            # plane-major copy of own-row weights: dual-fp8 LDWEIGHTS
            # rejects the byte-interleaved AP that the ifmap side accepts.
            nc.vector.tensor_copy(out=wpm[:], in_=zT8[:, :, 0:1024])

    # ---- phase B: Gram chunks + exp row-sums + bucket adds ----
    def bchunk(t, k):
        c0 = t * 128 + k * CH
        ps = ppool.tile([128, CH], F32, tag="ps", name="ps")
        for o, w in ((0, 512), (512, 512), (1024, 384)):
            nc.tensor.matmul(
                ps[:, o:o + w],
                wpm[:, :, t * 128:(t + 1) * 128],
                zT8[:, :, c0 + o:c0 + o + w],
                start=True, stop=True, perf_mode=DR,
            )
        e = epool.tile([128, CH], BF16, tag="e", name="e")
        nc.scalar.activation(
            e[:], ps[:], AF.Exp, bias=negtwo[:, 0:1], scale=QSCALE,
            accum_out=sparts[:, t * NCH + k:t * NCH + k + 1],
        )
        # bucket add: skip d=0 (self tile) and d=32 (rowsum-only) columns
        lo = max(c0, t * 128 + 128)
        hi = min(c0 + CH, t * 128 + 4096)
        if lo < hi:
            eng = nc.gpsimd if t in ADDS_GPS_T else nc.vector
            aw = acc[:, lo - 128:hi - 128]
            ew = e[:, lo - c0:hi - c0]
            eng.tensor_tensor(out=aw, in0=aw, in1=ew, op=ALU.add)

    # wave schedule: chunk (t,k) needs transposed tiles through t+... ;
    # req group = ((t*128 + (k+1)*CH - 1) // 128) // 8
    waves = {}
    for t in range(RT):
        for k in range(NCH):
            req = ((t * 128 + (k + 1) * CH - 1) // 128) // 8
            waves.setdefault(req, []).append((t, k))

    prev = None
    for g in range(NG):
        st = head(g)
        if prev is not None:
            tail(g - 1, prev)
            for (t, k) in waves.get(g - 1, ()):
                bchunk(t, k)
        prev = st
    tail(NG - 1, prev)
    for (t, k) in waves.get(NG - 1, ()):
        bchunk(t, k)

    # ---- phase C: self/pair dots on quantized rows ----
    for t in range(RT):
        nc.vector.scalar_tensor_tensor(
            out=junk_dot[:], in0=zq[:, t, :], scalar=1.0, in1=zq[:, t, :],
            op0=ALU.mult, op1=ALU.mult, accum_out=qss[:, t:t + 1],
        )
        nc.vector.scalar_tensor_tensor(
            out=junk_dot[:], in0=zq[:, t, :], scalar=1.0, in1=zq[:, t + 32, :],
            op0=ALU.mult, op1=ALU.mult,
            accum_out=outs_sb[:, 8 + t:9 + t],
        )

    # ---- phase D: finals ----
    nc.vector.tensor_reduce(
        Sown[:], sparts[:].rearrange("p (t k) -> p t k", k=NCH),
        axis=mybir.AxisListType.X, op=ALU.add,
    )
    nc.scalar.activation(ediag[:], qss[:], AF.Exp, bias=negtwo[:, 0:1], scale=QSCALE)
    nc.vector.tensor_sub(outs_sb[:, 0:8], Sown[:], ediag[:])
    nc.sync.dma_start_transpose(accT[:], acc[:])
    nc.vector.tensor_reduce(
        outc_sb[:], accT[:], axis=mybir.AxisListType.X, op=ALU.add,
    )
    nc.sync.dma_start(outs[:], outs_sb[:])
    nc.sync.dma_start(outc[:], outc_sb[:])


_CACHE = {}


def get_nc():
    if "nc" not in _CACHE:
        _patch_act_tables()
        nc = bacc.Bacc(
            "TRN2", target_bir_lowering=False, debug=False, num_devices=NCORES
        )
        z = nc.dram_tensor("z", [SR, D], F32, kind="ExternalInput").ap()
        outs = nc.dram_tensor("outs", [128, 16], F32, kind="ExternalOutput").ap()
        outc = nc.dram_tensor("outc", [128, 39], F32, kind="ExternalOutput").ap()
        with tile.TileContext(nc) as tc:
            with ExitStack() as ctx:
                _build_kernel(ctx, tc, z, outs, outc)
        nc.compile()
        _CACHE["nc"] = nc
    return _CACHE["nc"]


def make_in_maps(embeddings_i, embeddings_j):
    ei = np.ascontiguousarray(np.asarray(embeddings_i), dtype=np.float32)
    ej = np.ascontiguousarray(np.asarray(embeddings_j), dtype=np.float32)
    z = np.concatenate([ei, ej], axis=0)
    return [
        {"z": np.ascontiguousarray(np.roll(z, -ROWS * c, axis=0)[:SR])}
        for c in range(NCORES)
    ]


def reduce_results(results):
    S = np.zeros(N, np.float64)
    nmr = np.zeros(N, np.float64)
    for c, r in enumerate(results):
        os_ = r["outs"].astype(np.float64)
        oc = r["outc"].astype(np.float64)
        base = ROWS * c
        idx = (base + np.arange(ROWS)) % N
        S[idx] += os_[:, 0:8].T.reshape(-1)       # row = t*128 + p
        nmr[idx] = os_[:, 8:16].T.reshape(-1) * QSCALE
        idx2 = (base + 128 + np.arange(39 * 128)) % N
        S[idx2] += oc.T.reshape(-1)               # row = u*128 + p
    loss = np.mean(2.0 + np.log(S) - nmr)
    return np.float32(loss)


def run(embeddings_i, embeddings_j, **spmd_kwargs):
    nc = get_nc()
    in_maps = make_in_maps(embeddings_i, embeddings_j)
    res = run_bass_kernel_spmd(nc, in_maps, list(range(NCORES)), **spmd_kwargs)
    return reduce_results(res.results), res


def kernel(embeddings_i, embeddings_j):
    loss, _ = run(embeddings_i, embeddings_j)
    return loss
